# revision 6
# baseline (speedup 1.0000x reference)
"""Trainium2 Bass kernel for linear attention over external memory.

Computes out = x @ (keys^T @ vals) for
  x [4, 2048, 1024] f32, keys/vals [65536, 1024] f32.

Sharding across 8 NeuronCores: keys/vals sharded along the memory dim M
(8192 rows per core); each core computes a partial kv = keys_s^T @ vals_s,
AllReduces kv in bf16, then computes its token shard of x @ kv
(x sharded by token, 1024 rows per core).

Keys/vals are cast f32->bf16 during the DMA itself (SWDGE on the gpsimd
queue), so stage-2 matmuls run in bf16: the FWL fast weight load hides
the per-matmul LDWEIGHTS entirely (f32r weight loads of ~218ns were the
stage-2 bottleneck at 263ns/MM; bf16 streams at ~216ns/MM).

Bulk keys/vals tiles pack 4 chunks with a row-interleaved mapping
(partition p holds DRAM rows 4p..4p+3 = one contiguous 16KB segment), so
each 2MB cast-DMA needs only ~1 descriptor per partition. Each "virtual
chunk" t (rows = 4p+t) is a valid 128-row m-slice: keys and vals use the
same mapping, so the contraction pairing is preserved and kv = sum over
all m is order-independent.

The kv AllReduce is a single 2MB bf16 collective (or two 1MB halves with
BASS_AR_SPLIT=2): the final PSUM group drains fuse add+cast directly into
the bf16 bounce staging tile, bounce DMAs ride the scalar HWDGE queue,
and the PE fills the AllReduce wait with the x transposes (x is DMA'd at
the tail of the load stream where HBM is otherwise idle).
"""

import os

import numpy as np

# Problem shapes (hardcoded per contract).
B, S, D = 4, 2048, 1024
M = 65536
NCORES = 8
P = 128
T = (B * S) // NCORES          # 1024 tokens per core
KM = M // NCORES               # 8192 memory rows per core
NC_ = KM // P                  # 64 k-chunks
G = 8                          # chunks per PSUM accumulation group
NG = NC_ // G                  # 8 groups
DB = D // P                    # 8 d-blocks
HALF = D // 2                  # 512
TCH = T // P                   # 8 token chunks
QCH = 4                        # virtual chunks per packed tile
NTILE = NC_ // QCH             # 16 packed tiles (tile 0 loads as singles)

_CACHE = {}


def _build_nc(ar_split=1):
    import concourse.bacc as bacc
    import concourse.tile as tile
    from concourse import mybir
    from concourse.masks import make_identity

    f32 = mybir.dt.float32
    f32r = mybir.dt.float32r
    bf16 = mybir.dt.bfloat16
    ACT_COPY = mybir.ActivationFunctionType.Copy
    ADD = mybir.AluOpType.add

    nc = bacc.Bacc("TRN2", target_bir_lowering=False, debug=False,
                   num_devices=NCORES)

    xs_d = nc.dram_tensor("xs", [T, D], f32, kind="ExternalInput")
    ks_d = nc.dram_tensor("ks", [KM, D], f32r, kind="ExternalInput")
    vs_d = nc.dram_tensor("vs", [KM, D], f32r, kind="ExternalInput")
    out_d = nc.dram_tensor("out", [T, D], f32, kind="ExternalOutput")

    # Packed view: tile c, partition p holds rows c*512 + 4p + t for
    # t=0..3 (one 16KB contiguous DRAM segment per partition line).
    # Virtual chunk t of a tile is cols [t*1024, (t+1)*1024).
    ks_r4 = ks_d.ap().rearrange("(c p four) n -> c p (four n)", p=P, four=QCH)
    vs_r4 = vs_d.ap().rearrange("(c p four) n -> c p (four n)", p=P, four=QCH)
    # Classic per-chunk view for the f32r ramp chunks.
    ks_r1 = ks_d.ap().rearrange("(c p) n -> c p n", p=P)   # [64, 128, 1024]
    vs_r1 = vs_d.ap().rearrange("(c p) n -> c p n", p=P)
    xs_r = xs_d.ap().rearrange("(c p) n -> c p n", p=P)   # [8, 128, 1024]

    with tile.TileContext(nc) as tc:
        with (
            tc.tile_pool(name="const", bufs=1) as const,
            tc.tile_pool(name="kf1", bufs=QCH) as kf1,
            tc.tile_pool(name="vf1", bufs=QCH) as vf1,
            tc.tile_pool(name="kfp", bufs=4) as kfp,
            tc.tile_pool(name="vfp", bufs=4) as vfp,
            tc.tile_pool(name="accp", bufs=2 * DB) as accp,
            tc.tile_pool(name="xstage", bufs=6) as xstage,
            tc.tile_pool(name="xtp", bufs=DB) as xtp,
            tc.tile_pool(name="kvev", bufs=1) as kvevp,
            tc.tile_pool(name="kvh", bufs=2 * DB) as kvhp,
            tc.tile_pool(name="outp", bufs=3) as outp,
            tc.tile_pool(name="ps", bufs=8, space="PSUM") as ps,
            tc.tile_pool(name="dram", bufs=8, space="DRAM") as dram,
        ):
            # ---- ramp: first 4 chunks as f32r singles on the sync HWDGE
            # queue, which starts issuing ~4us before the SWDGE (gpsimd)
            # path wakes up. They join the same PSUM chains as the bf16
            # bulk chunks (PSUM accumulation is dtype-agnostic).
            ramp_k, ramp_v = [], []
            for t in range(QCH):
                kt = kf1.tile([P, D], f32r, name=f"k1_{t}", tag="k1")
                vt = vf1.tile([P, D], f32r, name=f"v1_{t}", tag="v1")
                nc.sync.dma_start(out=kt[:], in_=ks_r1[t])
                nc.sync.dma_start(out=vt[:], in_=vs_r1[t])
                ramp_k.append(kt)
                ramp_v.append(vt)

            # ---- bulk loads: 2MB f32 reads cast to 1MB bf16 tiles on
            # the gpsimd (SWDGE) queue. First tiles issue before the
            # warmup collective so data flows as early as possible.
            big_k = {}
            big_v = {}

            def load_big(b):
                kt = kfp.tile([P, QCH * D], bf16, name=f"kb{b}", tag="kb")
                vt = vfp.tile([P, QCH * D], bf16, name=f"vb{b}", tag="vb")
                nc.gpsimd.dma_start(out=kt[:], in_=ks_r4[b])
                nc.gpsimd.dma_start(out=vt[:], in_=vs_r4[b])
                big_k[b] = kt
                big_v[b] = vt

            for b in (1, 2, 3):
                load_big(b)

            # Warm-up collective: arms the ncfw collective stream so the
            # real AllReduce trigger doesn't pay the ~11us wake-up.
            warm = const.tile([P, 16], bf16)
            nc.gpsimd.memset(warm[:], 0.0)
            warm_in = dram.tile([P, 16], bf16, name="warm_in")
            warm_out = dram.tile([P, 16], bf16, name="warm_out",
                                 addr_space="Shared")
            nc.gpsimd.dma_start(out=warm_in[:], in_=warm[:])
            nc.gpsimd.collective_compute(
                "AllReduce",
                mybir.AluOpType.add,
                replica_groups=[list(range(NCORES))],
                ins=[warm_in.opt()],
                outs=[warm_out.opt()],
            )

            ident = const.tile([P, P], f32)
            make_identity(nc, ident)

            for b in range(4, NTILE):
                load_big(b)

            # x loads at the tail of the SWDGE ring: they drain after all
            # keys/vals traffic, landing during the last compute group /
            # AllReduce window when HBM is otherwise idle.
            xf_tiles = []
            for i in range(TCH):
                xf = xstage.tile([P, D], f32, name="xf", tag="xf")
                nc.gpsimd.dma_start(out=xf[:], in_=xs_r[i])
                xf_tiles.append(xf)

            def group_chunks(g):
                # list of (k_tile, v_tile, col_offset) for the 8 chunks
                if g == 0:
                    lst = [(ramp_k[t], ramp_v[t], 0) for t in range(QCH)]
                    lst += [(big_k[1], big_v[1], t * D) for t in range(QCH)]
                else:
                    lst = [(big_k[2 * g], big_v[2 * g], t * D)
                           for t in range(QCH)]
                    lst += [(big_k[2 * g + 1], big_v[2 * g + 1], t * D)
                            for t in range(QCH)]
                return lst

            # kv accumulator: tile (h*DB+j) holds kv[j*128:(j+1)*128,
            # h*512:(h+1)*512] as [128, 512] f32.
            acc = [accp.tile([P, HALF], f32, name=f"acc{i}", tag="acc")
                   for i in range(2 * DB)]

            # ---- stage 2, groups 0..NG-2: grouped PSUM accumulation ----
            for g in range(NG - 1):
                chunks = group_chunks(g)
                for h in range(2):
                    pst = [ps.tile([P, HALF], f32, name=f"kv{h}_{j}",
                                   tag="ps") for j in range(DB)]
                    for ci, (kt, vt, off) in enumerate(chunks):
                        for j in range(DB):
                            nc.tensor.matmul(
                                pst[j][:],
                                kt[:, off + j * P: off + (j + 1) * P],
                                vt[:, off + h * HALF: off + (h + 1) * HALF],
                                start=(ci == 0), stop=(ci == G - 1))
                    for j in range(DB):
                        if g == 0:
                            nc.vector.tensor_copy(out=acc[h * DB + j][:],
                                                  in_=pst[j][:])
                        else:
                            nc.vector.tensor_tensor(
                                out=acc[h * DB + j][:],
                                in0=pst[j][:],
                                in1=acc[h * DB + j][:],
                                op=ADD)

            # ---- final group: drains fuse add+cast straight into the
            # bf16 AllReduce staging tile so the collective fires ASAP.
            chunks = group_chunks(NG - 1)
            kvev = kvevp.tile([P, 2 * DB * HALF], bf16, name="kvev",
                              tag="kvev")
            if ar_split == 1:
                # j-major layout: cols (2j+h)*512. Process per-j bank
                # pairs; h1 drains ride gpsimd so the two halves' drains
                # run on parallel engines; bounce per j on scalar HWDGE.
                bounce_in = dram.tile([P, 2 * DB * HALF], bf16, name="bin",
                                      tag="bin")
                bounce_out = dram.tile([P, 2 * DB * HALF], bf16, name="bout",
                                       tag="bout", addr_space="Shared")
                for j in range(DB):
                    pa = ps.tile([P, HALF], f32, name=f"fka{j}", tag="ps")
                    pb = ps.tile([P, HALF], f32, name=f"fkb{j}", tag="ps")
                    for ci, (kt, vt, off) in enumerate(chunks):
                        nc.tensor.matmul(
                            pa[:], kt[:, off + j * P: off + (j + 1) * P],
                            vt[:, off: off + HALF],
                            start=(ci == 0), stop=(ci == G - 1))
                    for ci, (kt, vt, off) in enumerate(chunks):
                        nc.tensor.matmul(
                            pb[:], kt[:, off + j * P: off + (j + 1) * P],
                            vt[:, off + HALF: off + D],
                            start=(ci == 0), stop=(ci == G - 1))
                    sl_a = slice((2 * j) * HALF, (2 * j + 1) * HALF)
                    sl_b = slice((2 * j + 1) * HALF, (2 * j + 2) * HALF)
                    nc.vector.tensor_tensor(out=kvev[:, sl_a], in0=pa[:],
                                            in1=acc[0 * DB + j][:], op=ADD)
                    nc.vector.tensor_tensor(out=kvev[:, sl_b], in0=pb[:],
                                            in1=acc[1 * DB + j][:], op=ADD)
                    sl_j = slice(j * D, (j + 1) * D)
                    nc.scalar.dma_start(out=bounce_in[:, sl_j],
                                        in_=kvev[:, sl_j])
                nc.gpsimd.collective_compute(
                    "AllReduce",
                    mybir.AluOpType.add,
                    replica_groups=[list(range(NCORES))],
                    ins=[bounce_in.opt()],
                    outs=[bounce_out.opt()],
                )
                kv_src = {(h, j): (bounce_out, (2 * j + h) * HALF)
                          for h in range(2) for j in range(DB)}
            else:
                # h-major: two 1MB collectives; h0 fires a full half-group
                # of matmuls before stage-2 end.
                bouts = []
                for h in range(2):
                    bin_h = dram.tile([P, DB * HALF], bf16, name=f"bin{h}",
                                      tag="bin")
                    bout_h = dram.tile([P, DB * HALF], bf16, name=f"bout{h}",
                                       tag="bout", addr_space="Shared")
                    for j in range(DB):
                        pj = ps.tile([P, HALF], f32, name=f"fk{h}_{j}",
                                     tag="ps")
                        for ci, (kt, vt, off) in enumerate(chunks):
                            nc.tensor.matmul(
                                pj[:], kt[:, off + j * P: off + (j + 1) * P],
                                vt[:, off + h * HALF: off + (h + 1) * HALF],
                                start=(ci == 0), stop=(ci == G - 1))
                        sl = slice((h * DB + j) * HALF, (h * DB + j + 1) * HALF)
                        nc.vector.tensor_tensor(out=kvev[:, sl], in0=pj[:],
                                                in1=acc[h * DB + j][:], op=ADD)
                        nc.scalar.dma_start(
                            out=bin_h[:, j * HALF:(j + 1) * HALF],
                            in_=kvev[:, sl])
                    nc.gpsimd.collective_compute(
                        "AllReduce",
                        mybir.AluOpType.add,
                        replica_groups=[list(range(NCORES))],
                        ins=[bin_h.opt()],
                        outs=[bout_h.opt()],
                    )
                    bouts.append(bout_h)
                kv_src = {(h, j): (bouts[h], j * HALF)
                          for h in range(2) for j in range(DB)}

            # ---- x: PE-transpose fills the AllReduce wait ----
            xT = [xtp.tile([P, T], bf16, name=f"xT{j}", tag="xT")
                  for j in range(DB)]
            for i in range(TCH):
                xf = xf_tiles[i]
                for j in range(DB):
                    pt = ps.tile([P, P], f32, name="pt", tag="ps")
                    nc.tensor.transpose(
                        pt[:], xf[:, j * P:(j + 1) * P], ident[:])
                    nc.vector.tensor_copy(
                        out=xT[j][:, i * P:(i + 1) * P], in_=pt[:])

            # ---- stage 4: out = x @ kv ----
            kvh = {}
            for h in range(2):
                for j in range(DB):
                    kt = kvhp.tile([P, HALF], bf16, name=f"kvh{h}_{j}",
                                   tag="kvh")
                    src, off = kv_src[(h, j)]
                    nc.sync.dma_start(out=kt[:], in_=src[:, off:off + HALF])
                    kvh[(h, j)] = kt
            for h in range(2):
                for i in range(TCH):
                    po = ps.tile([P, HALF], f32, name="po", tag="ps")
                    for j in range(DB):
                        nc.tensor.matmul(
                            po[:],
                            xT[j][:, i * P:(i + 1) * P],
                            kvh[(h, j)][:],
                            start=(j == 0), stop=(j == DB - 1))
                    ob = outp.tile([P, HALF], f32, name="ob", tag="ob")
                    nc.scalar.activation(ob[:], po[:], ACT_COPY)
                    nc.scalar.dma_start(
                        out=out_d.ap()[i * P:(i + 1) * P,
                                       h * HALF:(h + 1) * HALF],
                        in_=ob[:])

    nc.compile()
    return nc


def _get_nc():
    ar_split = int(os.environ.get("BASS_AR_SPLIT", "2"))
    key = ("nc", ar_split)
    if key not in _CACHE:
        _CACHE[key] = _build_nc(ar_split)
    return _CACHE[key]


def kernel(**inputs):
    from concourse.bass_utils import run_bass_kernel_spmd

    x = np.ascontiguousarray(np.asarray(inputs["x"], dtype=np.float32))
    keys = np.ascontiguousarray(np.asarray(inputs["keys"], dtype=np.float32))
    vals = np.ascontiguousarray(np.asarray(inputs["vals"], dtype=np.float32))
    xf = x.reshape(B * S, D)

    nc = _get_nc()
    in_maps = []
    for c in range(NCORES):
        in_maps.append({
            "xs": xf[c * T:(c + 1) * T],
            "ks": keys[c * KM:(c + 1) * KM],
            "vs": vals[c * KM:(c + 1) * KM],
        })
    res = run_bass_kernel_spmd(nc, in_maps, list(range(NCORES)))
    out = np.concatenate([res.results[c]["out"] for c in range(NCORES)],
                         axis=0)
    return out.reshape(B, S, D).astype(np.float32)


# revision 7
# speedup vs baseline: 1.1253x; 1.1253x over previous
"""Trainium2 Bass kernel for linear attention over external memory.

Computes out = x @ (keys^T @ vals) for
  x [4, 2048, 1024] f32, keys/vals [65536, 1024] f32.

Sharding across 8 NeuronCores: keys/vals sharded along the memory dim M
(8192 rows per core); each core computes a partial kv = keys_s^T @ vals_s,
AllReduces kv in bf16 (two 1MB column-half collectives), then computes
its token shard of x @ kv (x sharded by token, 1024 rows per core).

Stage 2 runs f32r (full PE rate at moving dim 512; the PE sustains
~263ns per 128x128x512 matmul under load regardless of input dtype).
Keys stream on the sync HWDGE queue, vals on the scalar HWDGE queue
(two independent rings), packed 2 chunks per 1MB transfer with a
row-interleaved mapping (partition p holds DRAM rows 2p, 2p+1 = one
contiguous 8KB segment). Each "virtual chunk" t is a valid 128-row
m-slice since keys and vals use identical mappings and kv sums over
all of m order-independently.

Collective hiding: the last two chunk-groups are processed h0-first
(g6-h0, g7-h0, then g6-h1, g7-h1), so the h0 AllReduce fires ~31us
before stage-2 ends and completes roughly when the PE finishes; the
h1 AllReduce's latency is then covered by the x transposes plus
stage-4 h0. The kv accumulator lives in bf16 (also the AllReduce
payload dtype), so final drains bounce straight to DRAM with no cast
step. x loads ride at the tail of the sync queue and land during the
AllReduce window.
"""

import os

import numpy as np

# Problem shapes (hardcoded per contract).
B, S, D = 4, 2048, 1024
M = 65536
NCORES = 8
P = 128
T = (B * S) // NCORES          # 1024 tokens per core
KM = M // NCORES               # 8192 memory rows per core
NC_ = KM // P                  # 64 k-chunks
G = 8                          # chunks per PSUM accumulation group
NG = NC_ // G                  # 8 groups
DB = D // P                    # 8 d-blocks
HALF = D // 2                  # 512
TCH = T // P                   # 8 token chunks
TWO = 2                        # virtual chunks per packed tile
NTILE = NC_ // TWO             # 32 packed tiles
TPG = G // TWO                 # 4 tiles per group

_CACHE = {}


def _build_nc():
    import concourse.bacc as bacc
    import concourse.tile as tile
    from concourse import mybir
    from concourse.masks import make_identity

    f32 = mybir.dt.float32
    f32r = mybir.dt.float32r
    bf16 = mybir.dt.bfloat16
    ACT_COPY = mybir.ActivationFunctionType.Copy
    ADD = mybir.AluOpType.add

    nc = bacc.Bacc("TRN2", target_bir_lowering=False, debug=False,
                   num_devices=NCORES)

    xs_d = nc.dram_tensor("xs", [T, D], f32, kind="ExternalInput")
    ks_d = nc.dram_tensor("ks", [KM, D], f32r, kind="ExternalInput")
    vs_d = nc.dram_tensor("vs", [KM, D], f32r, kind="ExternalInput")
    out_d = nc.dram_tensor("out", [T, D], f32, kind="ExternalOutput")

    # Packed view: tile c, partition p holds rows c*256 + 2p + t for
    # t in {0,1} (one 8KB contiguous DRAM segment per partition line).
    # Virtual chunk t of a tile is cols [t*1024, (t+1)*1024).
    ks_r2 = ks_d.ap().rearrange("(c p two) n -> c p (two n)", p=P, two=TWO)
    vs_r2 = vs_d.ap().rearrange("(c p two) n -> c p (two n)", p=P, two=TWO)
    xs_r = xs_d.ap().rearrange("(c p) n -> c p n", p=P)   # [8, 128, 1024]

    with tile.TileContext(nc) as tc:
        with (
            tc.tile_pool(name="const", bufs=1) as const,
            tc.tile_pool(name="kfp", bufs=8) as kfp,
            tc.tile_pool(name="vfp", bufs=8) as vfp,
            tc.tile_pool(name="accp", bufs=2 * DB) as accp,
            tc.tile_pool(name="xstage", bufs=5) as xstage,
            tc.tile_pool(name="xtp", bufs=DB) as xtp,
            tc.tile_pool(name="kvh", bufs=2 * DB) as kvhp,
            tc.tile_pool(name="outp", bufs=3) as outp,
            tc.tile_pool(name="ps", bufs=8, space="PSUM") as ps,
            tc.tile_pool(name="dram", bufs=8, space="DRAM") as dram,
        ):
            # ---- input streams: keys on sync, vals on scalar (separate
            # HWDGE rings), 1MB packed transfers.
            ktiles, vtiles = [], []
            for b in range(NTILE):
                kt = kfp.tile([P, TWO * D], f32r, name=f"kb{b}", tag="kb")
                vt = vfp.tile([P, TWO * D], f32r, name=f"vb{b}", tag="vb")
                nc.sync.dma_start(out=kt[:], in_=ks_r2[b])
                nc.scalar.dma_start(out=vt[:], in_=vs_r2[b])
                ktiles.append(kt)
                vtiles.append(vt)

            # x loads at the tail of the sync stream: they drain after
            # all keys traffic, landing during the last compute groups
            # when HBM is otherwise winding down.
            xf_tiles = []
            for i in range(TCH):
                xf = xstage.tile([P, D], f32, name="xf", tag="xf")
                nc.sync.dma_start(out=xf[:], in_=xs_r[i])
                xf_tiles.append(xf)

            # Warm-up collective: arms the ncfw collective stream so the
            # first real AllReduce trigger doesn't pay the ~50us wake-up.
            warm = const.tile([P, 16], bf16)
            nc.gpsimd.memset(warm[:], 0.0)
            warm_in = dram.tile([P, 16], bf16, name="warm_in")
            warm_out = dram.tile([P, 16], bf16, name="warm_out",
                                 addr_space="Shared")
            nc.gpsimd.dma_start(out=warm_in[:], in_=warm[:])
            nc.gpsimd.collective_compute(
                "AllReduce",
                mybir.AluOpType.add,
                replica_groups=[list(range(NCORES))],
                ins=[warm_in.opt()],
                outs=[warm_out.opt()],
            )

            ident = const.tile([P, P], f32)
            make_identity(nc, ident)

            def chunks_of(g):
                # (k_tile, v_tile, col_offset) for the 8 chunks of group g
                return [(ktiles[TPG * g + b], vtiles[TPG * g + b], t * D)
                        for b in range(TPG) for t in range(TWO)]

            # kv accumulator in bf16: tile (h*DB+j) holds
            # kv[j*128:(j+1)*128, h*512:(h+1)*512] as [128, 512].
            acc = [accp.tile([P, HALF], bf16, name=f"acc{i}", tag="acc")
                   for i in range(2 * DB)]

            def mm_chain(pst, chunks, h, j):
                for ci, (kt, vt, off) in enumerate(chunks):
                    nc.tensor.matmul(
                        pst[:],
                        kt[:, off + j * P: off + (j + 1) * P],
                        vt[:, off + h * HALF: off + (h + 1) * HALF],
                        start=(ci == 0), stop=(ci == G - 1))

            # ---- stage 2, groups 0..5: c-outer chains (compute starts
            # as soon as each chunk lands), drains into the accumulator.
            for g in range(NG - 2):
                chunks = chunks_of(g)
                for h in range(2):
                    pst = [ps.tile([P, HALF], f32, name=f"kv{h}_{j}",
                                   tag="ps") for j in range(DB)]
                    for ci, (kt, vt, off) in enumerate(chunks):
                        for j in range(DB):
                            nc.tensor.matmul(
                                pst[j][:],
                                kt[:, off + j * P: off + (j + 1) * P],
                                vt[:, off + h * HALF: off + (h + 1) * HALF],
                                start=(ci == 0), stop=(ci == G - 1))
                    for j in range(DB):
                        if g == 0:
                            nc.vector.tensor_copy(out=acc[h * DB + j][:],
                                                  in_=pst[j][:])
                        else:
                            nc.vector.tensor_tensor(
                                out=acc[h * DB + j][:],
                                in0=pst[j][:],
                                in1=acc[h * DB + j][:],
                                op=ADD)

            # ---- groups 6+7, h0 first: the h0 AllReduce fires ~31us
            # before stage-2 ends and hides behind the h1 compute.
            cg6, cg7 = chunks_of(NG - 2), chunks_of(NG - 1)
            bouts = []
            collectives = []
            for h in range(2):
                bin_h = dram.tile([P, DB * HALF], bf16, name=f"bin{h}",
                                  tag="bin")
                bout_h = dram.tile([P, DB * HALF], bf16, name=f"bout{h}",
                                   tag="bout", addr_space="Shared")
                # j-outer: chain j completes early, so drains and bounce
                # DMAs pipeline behind the remaining chains.
                for j in range(DB):
                    p6 = ps.tile([P, HALF], f32, name=f"g6_{h}_{j}",
                                 tag="ps")
                    mm_chain(p6, cg6, h, j)
                    nc.vector.tensor_tensor(out=acc[h * DB + j][:],
                                            in0=p6[:],
                                            in1=acc[h * DB + j][:], op=ADD)
                for j in range(DB):
                    p7 = ps.tile([P, HALF], f32, name=f"g7_{h}_{j}",
                                 tag="ps")
                    mm_chain(p7, cg7, h, j)
                    nc.vector.tensor_tensor(out=acc[h * DB + j][:],
                                            in0=p7[:],
                                            in1=acc[h * DB + j][:], op=ADD)
                    # acc is bf16 == the collective payload dtype: bounce
                    # straight out, no cast step. gpsimd ring is idle.
                    nc.gpsimd.dma_start(
                        out=bin_h[:, j * HALF:(j + 1) * HALF],
                        in_=acc[h * DB + j][:])
                nc.gpsimd.collective_compute(
                    "AllReduce",
                    mybir.AluOpType.add,
                    replica_groups=[list(range(NCORES))],
                    ins=[bin_h.opt()],
                    outs=[bout_h.opt()],
                )
                bouts.append(bout_h)

            # ---- x: PE-transpose fills the AllReduce wait ----
            xT = [xtp.tile([P, T], bf16, name=f"xT{j}", tag="xT")
                  for j in range(DB)]
            for i in range(TCH):
                xf = xf_tiles[i]
                for j in range(DB):
                    pt = ps.tile([P, P], f32, name="pt", tag="ps")
                    nc.tensor.transpose(
                        pt[:], xf[:, j * P:(j + 1) * P], ident[:])
                    nc.vector.tensor_copy(
                        out=xT[j][:, i * P:(i + 1) * P], in_=pt[:])

            # ---- stage 4: out = x @ kv ----
            kvh = {}
            for h in range(2):
                for j in range(DB):
                    kt = kvhp.tile([P, HALF], bf16, name=f"kvh{h}_{j}",
                                   tag="kvh")
                    nc.sync.dma_start(
                        out=kt[:],
                        in_=bouts[h][:, j * HALF:(j + 1) * HALF])
                    kvh[(h, j)] = kt
            for h in range(2):
                for i in range(TCH):
                    po = ps.tile([P, HALF], f32, name="po", tag="ps")
                    for j in range(DB):
                        nc.tensor.matmul(
                            po[:],
                            xT[j][:, i * P:(i + 1) * P],
                            kvh[(h, j)][:],
                            start=(j == 0), stop=(j == DB - 1))
                    ob = outp.tile([P, HALF], f32, name="ob", tag="ob")
                    nc.scalar.activation(ob[:], po[:], ACT_COPY)
                    nc.scalar.dma_start(
                        out=out_d.ap()[i * P:(i + 1) * P,
                                       h * HALF:(h + 1) * HALF],
                        in_=ob[:])

    nc.compile()
    return nc


def _get_nc():
    if "nc" not in _CACHE:
        _CACHE["nc"] = _build_nc()
    return _CACHE["nc"]


def kernel(**inputs):
    from concourse.bass_utils import run_bass_kernel_spmd

    x = np.ascontiguousarray(np.asarray(inputs["x"], dtype=np.float32))
    keys = np.ascontiguousarray(np.asarray(inputs["keys"], dtype=np.float32))
    vals = np.ascontiguousarray(np.asarray(inputs["vals"], dtype=np.float32))
    xf = x.reshape(B * S, D)

    nc = _get_nc()
    in_maps = []
    for c in range(NCORES):
        in_maps.append({
            "xs": xf[c * T:(c + 1) * T],
            "ks": keys[c * KM:(c + 1) * KM],
            "vs": vals[c * KM:(c + 1) * KM],
        })
    res = run_bass_kernel_spmd(nc, in_maps, list(range(NCORES)))
    out = np.concatenate([res.results[c]["out"] for c in range(NCORES)],
                         axis=0)
    return out.reshape(B, S, D).astype(np.float32)
